# revision 1
# baseline (speedup 1.0000x reference)
"""GatedMemoryTitan kernel for 8 NeuronCores (TRN2, Bass/Tile).

Sharding: core c -> batch b=c//2, sequence half h=c%2 (1024 query rows each).
No collectives: each core holds the full combined sequence for its batch,
computes its 1024 output rows; the host gathers.

Layout conventions on-chip:
  *_fm  "feature-major": [feature (128-partition chunks), tokens]
  *_tm  "token-major":   [tokens (128-partition tiles), features]
All matmuls run in bf16 with f32 PSUM accumulation. Softmax is computed on
transposed scores (exp only -- scores are tiny by construction, no max
subtraction needed); the normalizer comes from an appended ones-column in the
value matrix. Biases are folded in as rank-1 matmul updates (ones row trick).
"""

import numpy as np
import ml_dtypes

BF16 = ml_dtypes.bfloat16

D, H, HD, PM, S0, B = 1024, 16, 64, 32, 2016, 4
S = PM + S0            # 2048
NC_ = 8
QH = S // 2            # 1024 queries per core
WIN = 256              # structural window (masks use the runtime value)
KV = WIN + QH          # 1280-token kv range per core (left-padded)
NQT = QH // 128        # 8 query tiles
NFC = D // 128         # 8 feature chunks
NKT = S // 128         # 16 key tiles (memory attention)
NVT = KV // 128        # 10 value token tiles (SWA)

_CACHE = {}


def _build_program():
    import concourse.bass as bass
    import concourse.bacc as bacc
    import concourse.mybir as mybir
    import concourse.tile as tile
    from contextlib import ExitStack

    dt = mybir.dt
    f32, bf16 = dt.float32, dt.bfloat16
    AF = mybir.ActivationFunctionType
    AL = mybir.AluOpType
    nc = bacc.Bacc("TRN2", target_bir_lowering=False)

    def inp(name, shape, dtype=bf16):
        return nc.dram_tensor(name, shape, dtype, kind="ExternalInput")

    cfT = inp("cfT", [D, S])
    ckT = inp("ckT", [D, KV])
    wq = inp("wq", [D, D])
    wk = inp("wk", [D, D])
    wv = inp("wv", [D, D])
    wo = inp("wo", [D, D])
    mwq = inp("mwq", [D, D])
    mwk = inp("mwk", [D, D])
    mwv = inp("mwv", [D, D])
    gw1 = inp("gw1", [D, D])
    gw2 = inp("gw2", [D, D])
    brows = inp("brows", [1, 8 * D])     # bq bk bv bo mbq mbk mbv gate_b
    gvecs = inp("gvecs", [4, D])     # g1_w g1_b g2_w g2_b
    masks = inp("masks", [128, NQT * 384])
    ident = inp("ident", [128, 128])
    onesr = inp("onesr", [1, 512])
    out = nc.dram_tensor("out", [QH, D], f32, kind="ExternalOutput")
    out_na = nc.dram_tensor("out_na", [QH, D], f32, kind="ExternalOutput")
    out_nm = nc.dram_tensor("out_nm", [QH, D], f32, kind="ExternalOutput")
    out_q = nc.dram_tensor("out_q", [D, QH], bf16, kind="ExternalOutput")
    out_k = nc.dram_tensor("out_k", [D, KV], bf16, kind="ExternalOutput")
    out_ao1 = nc.dram_tensor("out_ao1", [QH, D], bf16, kind="ExternalOutput")

    def chunked(ap):  # dram [D, N] -> [128, NFC, N]
        return ap[:, :].rearrange("(c p) n -> p c n", p=128)

    ctx = ExitStack()
    with tile.TileContext(nc) as tc, ctx:

        def pool_enter(**kw):
            cm = tc.tile_pool(**kw)
            return cm, cm.__enter__()

        def pool_exit(cm):
            cm.__exit__(None, None, None)

        # ------------- small constants (live whole kernel) -------------
        persist = ctx.enter_context(tc.tile_pool(name="persist", bufs=1))
        id_sb = persist.tile([128, 128], bf16)
        nc.sync.dma_start(out=id_sb, in_=ident[:, :])
        ones_sb = persist.tile([1, 512], bf16)
        nc.sync.dma_start(out=ones_sb, in_=onesr[:, :])
        brow_sb = persist.tile([1, 8 * D], bf16)
        nc.sync.dma_start(out=brow_sb, in_=brows[:, :])
        eps_sb = persist.tile([128, 1], f32)
        nc.vector.memset(eps_sb, 1e-5)

        def load_w(pool, w, tag="wrot"):
            t = pool.tile([128, NFC, D], bf16, tag=tag)
            nc.sync.dma_start(out=t, in_=chunked(w))
            return t

        def proj_fm(pp, w_sb, src_sb, dst_sb, brow_idx, groups, src_off=0):
            for mc in range(NFC):
                for g0, gn in groups:
                    ps = pp.tile([128, 512], f32, tag="pj")
                    for kc in range(NFC):
                        nc.tensor.matmul(
                            ps[:, :gn], lhsT=w_sb[:, kc, mc * 128:(mc + 1) * 128],
                            rhs=src_sb[:, kc, src_off + g0: src_off + g0 + gn],
                            start=(kc == 0), stop=False)
                    nc.tensor.matmul(
                        ps[:, :gn],
                        lhsT=brow_sb[0:1, brow_idx * D + mc * 128:
                                     brow_idx * D + (mc + 1) * 128],
                        rhs=ones_sb[0:1, 0:gn], start=False, stop=True)
                    nc.scalar.copy(out=dst_sb[:, mc, g0:g0 + gn], in_=ps[:, :gn])

        def proj_tm_aug(pp, w_sb, src_sb, brow_idx, ntile, scatter):
            for tt in range(ntile):
                for ng in range(2):
                    ps = pp.tile([128, 512], f32, tag="pj")
                    for kc in range(NFC):
                        nc.tensor.matmul(
                            ps, lhsT=src_sb[:, kc, tt * 128:(tt + 1) * 128],
                            rhs=w_sb[:, kc, ng * 512:(ng + 1) * 512],
                            start=(kc == 0), stop=False)
                    nc.tensor.matmul(
                        ps, lhsT=ones_sb[0:1, 0:128],
                        rhs=brow_sb[0:1, brow_idx * D + ng * 512:
                                    brow_idx * D + (ng + 1) * 512],
                        start=False, stop=True)
                    nc.scalar.copy(out=scatter(tt, ng), in_=ps)

        # ---- phase A: mem q/k projections ----
        ckp_cm, ck_pool = pool_enter(name="ckp", bufs=1)
        ck_sb = ck_pool.tile([128, NFC, KV], bf16)
        nc.sync.dma_start(out=ck_sb, in_=chunked(ckT))
        cfp_cm, cf_pool = pool_enter(name="cfp", bufs=1)
        cf_sb = cf_pool.tile([128, NFC, S], bf16)
        nc.sync.dma_start(out=cf_sb, in_=chunked(cfT))
        mqk_cm, mqk_pool = pool_enter(name="mqk", bufs=1)
        mq_sb = mqk_pool.tile([128, NFC, QH], bf16)
        mk_sb = mqk_pool.tile([128, NFC, S], bf16)
        with tc.tile_pool(name="wrot1", bufs=2) as wpool, \
             tc.tile_pool(name="pp1", bufs=4, space="PSUM") as pp:
            w_sb = load_w(wpool, mwq, tag="wrot1")
            proj_fm(pp, w_sb, ck_sb, mq_sb, 4, [(0, 512), (512, 512)],
                    src_off=WIN)
            w_sb = load_w(wpool, mwk, tag="wrot1")
            proj_fm(pp, w_sb, cf_sb, mk_sb, 5,
                    [(g * 512, 512) for g in range(4)])

        # ---- phase B: memory scores -> exp ----
        matp_cm, mat_pool = pool_enter(name="matp", bufs=1, side="right")
        mat_sb = mat_pool.tile([128, NKT, QH], bf16)
        with tc.tile_pool(name="pms", bufs=4, space="PSUM") as pms_p:
            for kt in range(NKT):
                for qg in range(2):
                    pm = pms_p.tile([128, 512], f32, tag="pm")
                    for fc in range(NFC):
                        nc.tensor.matmul(
                            pm, lhsT=mk_sb[:, fc, kt * 128:(kt + 1) * 128],
                            rhs=mq_sb[:, fc, qg * 512:(qg + 1) * 512],
                            start=(fc == 0), stop=(fc == NFC - 1))
                    nc.scalar.activation(
                        out=mat_sb[:, kt, qg * 512:(qg + 1) * 512],
                        in_=pm, func=AF.Exp)
        pool_exit(mqk_cm)

        # ---- phase C: mv projection ----
        mvp_cm, mv_pool = pool_enter(name="mvp", bufs=1, side="right")
        mv_sb = mv_pool.tile([128, NKT, 1025], bf16)
        with tc.tile_pool(name="wrot1b", bufs=2) as wpool, \
             tc.tile_pool(name="pp1b", bufs=4, space="PSUM") as pp:
            nc.vector.memset(mv_sb[:, :, 1024:1025], 1.0)
            w_sb = load_w(wpool, mwv, tag="wrot1b")
            proj_tm_aug(pp, w_sb, cf_sb, 6, NKT,
                        lambda tt, ng: mv_sb[:, tt, ng * 512:(ng + 1) * 512])
        pool_exit(cfp_cm)

        # ---- phase D: SWA projections ----
        swa_cm, swa_pool = pool_enter(name="swa", bufs=1, side="right")
        q_sb = swa_pool.tile([128, NFC, QH], bf16)
        k_sb = swa_pool.tile([128, NFC, KV], bf16)
        v_sb = swa_pool.tile([128, NVT, H, 65], bf16)
        with tc.tile_pool(name="wrot2", bufs=2) as wpool, \
             tc.tile_pool(name="pp2", bufs=4, space="PSUM") as pp:
            nc.vector.memset(v_sb[:, :, :, 64:65], 1.0)
            w_sb = load_w(wpool, wq, tag="wrot2")
            proj_fm(pp, w_sb, ck_sb, q_sb, 0, [(0, 512), (512, 512)],
                    src_off=WIN)
            w_sb = load_w(wpool, wk, tag="wrot2")
            proj_fm(pp, w_sb, ck_sb, k_sb, 1,
                    [(0, 512), (512, 512), (1024, 256)])
            w_sb = load_w(wpool, wv, tag="wrot2")
            proj_tm_aug(pp, w_sb, ck_sb, 2, NVT,
                        lambda tt, ng: v_sb[:, tt, ng * 8:(ng + 1) * 8, 0:64])
        for fc in range(NFC):
            nc.sync.dma_start(out=out_q[fc * 128:(fc + 1) * 128, :],
                              in_=q_sb[:, fc, :])
            nc.sync.dma_start(out=out_k[fc * 128:(fc + 1) * 128, :],
                              in_=k_sb[:, fc, :])
        pool_exit(ckp_cm)

        # ---- phase E: SWA attention -> ao1 (feature-major) ----
        wo_pool = ctx.enter_context(tc.tile_pool(name="wop", bufs=1))
        wo_sb = load_w(wo_pool, wo, tag="wo")
        ao1f_sb = wo_pool.tile([128, NFC, QH], bf16, tag="ao1f")
        with tc.tile_pool(name="maskp", bufs=1) as mask_pool, \
             tc.tile_pool(name="psc", bufs=2, space="PSUM") as psc_p, \
             tc.tile_pool(name="pav", bufs=1, space="PSUM") as pav_p, \
             tc.tile_pool(name="ptp", bufs=2, space="PSUM") as ptp_p, \
             tc.tile_pool(name="sat", bufs=4) as sat_p, \
             tc.tile_pool(name="sao", bufs=2) as sao_p:
            mask_sb = mask_pool.tile([128, NQT * 384], bf16)
            nc.sync.dma_start(out=mask_sb, in_=masks[:, :])
            for t in range(NQT):
                pav = pav_p.tile([128, H, 128], f32, tag="av")
                for h in range(H):
                    hp, hr = h // 2, (h % 2) * 64
                    psc = psc_p.tile([128, 384], f32, tag="sc")
                    for c in range(3):
                        nc.tensor.matmul(
                            psc[:, c * 128:(c + 1) * 128],
                            lhsT=k_sb[hr:hr + 64, hp,
                                      (t + c) * 128:(t + c + 1) * 128],
                            rhs=q_sb[hr:hr + 64, hp, t * 128:(t + 1) * 128],
                            start=True, stop=True)
                    at = sat_p.tile([128, 384], bf16, tag="at")
                    nc.scalar.activation(out=at, in_=psc, func=AF.Exp)
                    nc.vector.tensor_mul(
                        at, at, mask_sb[:, t * 384:(t + 1) * 384])
                    for c in range(3):
                        nc.tensor.matmul(
                            pav[:, h, 0:65], lhsT=at[:, c * 128:(c + 1) * 128],
                            rhs=v_sb[:, t + c, h, :],
                            start=(c == 0), stop=(c == 2))
                rec = sat_p.tile([128, 16], f32, tag="rec")
                nc.vector.reciprocal(rec, pav[:, :, 64:65])
                ra = rec[:, :]
                ao1 = sao_p.tile([128, 1024], bf16, tag="ao1")
                nc.vector.tensor_tensor(
                    out=ao1, in0=pav[:, :, 0:64],
                    in1=bass.AP(tensor=ra.tensor, offset=ra.offset,
                                ap=[ra.ap[0], ra.ap[1], [0, 64]]),
                    op=AL.mult)
                nc.sync.dma_start(out=out_ao1[t * 128:(t + 1) * 128, :],
                                  in_=ao1)
                for fc in range(NFC):
                    ptp = ptp_p.tile([128, 128], bf16, tag="tp")
                    nc.tensor.transpose(
                        ptp, ao1[:, fc * 128:(fc + 1) * 128], id_sb)
                    nc.scalar.copy(
                        out=ao1f_sb[:, fc, t * 128:(t + 1) * 128], in_=ptp)
        pool_exit(swa_cm)

        # ---- phase F: Wo + layernorm -> na ----
        nap = ctx.enter_context(tc.tile_pool(name="nap", bufs=1))
        gb_sb = nap.tile([128, 4, D], bf16, tag="gb")
        gv = gvecs[:, :]
        nc.sync.dma_start(
            out=gb_sb,
            in_=bass.AP(tensor=gv.tensor, offset=gv.offset,
                        ap=[[0, 128], gv.ap[0], gv.ap[1]]))
        na_sb = nap.tile([128, NQT, D], bf16, tag="na")
        nm_sb = nap.tile([128, NQT, D], bf16, tag="nm")
        with tc.tile_pool(name="pa3", bufs=4, space="PSUM") as pa_p, \
             tc.tile_pool(name="st3", bufs=4) as st_p:
            for t in range(NQT):
                pas = []
                for ng in range(2):
                    pa = pa_p.tile([128, 512], f32, tag="pa")
                    for fc in range(NFC):
                        nc.tensor.matmul(
                            pa, lhsT=ao1f_sb[:, fc, t * 128:(t + 1) * 128],
                            rhs=wo_sb[:, fc, ng * 512:(ng + 1) * 512],
                            start=(fc == 0), stop=False)
                    nc.tensor.matmul(
                        pa, lhsT=ones_sb[0:1, 0:128],
                        rhs=brow_sb[0:1, 3 * D + ng * 512:
                                    3 * D + (ng + 1) * 512],
                        start=False, stop=True)
                    pas.append(pa)
                st = st_p.tile([128, 2, 6], f32, tag="st")
                for ng in range(2):
                    nc.vector.bn_stats(st[:, ng, :], pas[ng])
                mv_ = st_p.tile([128, 2], f32, tag="mv")
                nc.vector.bn_aggr(mv_, st)
                r1 = st_p.tile([128, 1], f32, tag="r1")
                nc.scalar.activation(out=r1, in_=mv_[:, 1:2], func=AF.Sqrt,
                                     bias=eps_sb, scale=1.0)
                nc.vector.reciprocal(r1, r1)
                for ng in range(2):
                    tmp = st_p.tile([128, 512], bf16, tag="tmp")
                    nc.vector.scalar_tensor_tensor(
                        out=tmp, in0=pas[ng], scalar=mv_[:, 0:1],
                        in1=gb_sb[:, 0, ng * 512:(ng + 1) * 512],
                        op0=AL.subtract, op1=AL.mult)
                    nc.vector.scalar_tensor_tensor(
                        out=na_sb[:, t, ng * 512:(ng + 1) * 512],
                        in0=tmp, scalar=r1,
                        in1=gb_sb[:, 1, ng * 512:(ng + 1) * 512],
                        op0=AL.mult, op1=AL.add)

        # ---- phase G: memory attn@v + layernorm -> nm ----
        with tc.tile_pool(name="pmo", bufs=1, space="PSUM") as pmo_p, \
             tc.tile_pool(name="st5", bufs=4) as st_p:
            for t in range(NQT):
                pmo = pmo_p.tile([128, 1025], f32, tag="mo")
                for kc in range(NKT):
                    for ng in range(2):
                        nc.tensor.matmul(
                            pmo[:, ng * 512:(ng + 1) * 512],
                            lhsT=mat_sb[:, kc, t * 128:(t + 1) * 128],
                            rhs=mv_sb[:, kc, ng * 512:(ng + 1) * 512],
                            start=(kc == 0), stop=(kc == NKT - 1))
                    nc.tensor.matmul(
                        pmo[:, 1024:1025],
                        lhsT=mat_sb[:, kc, t * 128:(t + 1) * 128],
                        rhs=mv_sb[:, kc, 1024:1025],
                        start=(kc == 0), stop=(kc == NKT - 1))
                rm = st_p.tile([128, 1], f32, tag="rm")
                nc.vector.reciprocal(rm, pmo[:, 1024:1025])
                mo = st_p.tile([128, 1024], bf16, tag="mo_sb")
                nc.vector.tensor_scalar(
                    out=mo, in0=pmo[:, 0:1024], scalar1=rm, scalar2=None,
                    op0=AL.mult)
                st = st_p.tile([128, 2, 6], f32, tag="st5")
                for ng in range(2):
                    nc.vector.bn_stats(
                        st[:, ng, :], mo[:, ng * 512:(ng + 1) * 512])
                mv_ = st_p.tile([128, 2], f32, tag="mv5")
                nc.vector.bn_aggr(mv_, st)
                r2 = st_p.tile([128, 1], f32, tag="r2")
                nc.scalar.activation(out=r2, in_=mv_[:, 1:2], func=AF.Sqrt,
                                     bias=eps_sb, scale=1.0)
                nc.vector.reciprocal(r2, r2)
                for ng in range(2):
                    tmp = st_p.tile([128, 512], bf16, tag="tmp5")
                    nc.vector.scalar_tensor_tensor(
                        out=tmp, in0=mo[:, ng * 512:(ng + 1) * 512],
                        scalar=mv_[:, 0:1],
                        in1=gb_sb[:, 2, ng * 512:(ng + 1) * 512],
                        op0=AL.subtract, op1=AL.mult)
                    nc.vector.scalar_tensor_tensor(
                        out=nm_sb[:, t, ng * 512:(ng + 1) * 512],
                        in0=tmp, scalar=r2,
                        in1=gb_sb[:, 3, ng * 512:(ng + 1) * 512],
                        op0=AL.mult, op1=AL.add)
        pool_exit(mvp_cm)
        pool_exit(matp_cm)

        # ---- phase H: gate + final combine ----
        with tc.tile_pool(name="gw", bufs=1) as gw_pool, \
             tc.tile_pool(name="ptp6", bufs=4, space="PSUM") as ptp_p, \
             tc.tile_pool(name="pg6", bufs=2, space="PSUM") as pg_p, \
             tc.tile_pool(name="s6", bufs=3) as s6_p:
            gw1_sb = gw_pool.tile([128, NFC, D], bf16, tag="g1")
            nc.sync.dma_start(out=gw1_sb, in_=chunked(gw1))
            gw2_sb = gw_pool.tile([128, NFC, D], bf16, tag="g2")
            nc.sync.dma_start(out=gw2_sb, in_=chunked(gw2))
            for t in range(NQT):
                naf = s6_p.tile([128, NFC, 128], bf16, tag="naf")
                nmf = s6_p.tile([128, NFC, 128], bf16, tag="nmf")
                for fc in range(NFC):
                    ptp = ptp_p.tile([128, 128], bf16, tag="tp6")
                    nc.tensor.transpose(
                        ptp, na_sb[:, t, fc * 128:(fc + 1) * 128], id_sb)
                    nc.scalar.copy(out=naf[:, fc, :], in_=ptp)
                    ptp = ptp_p.tile([128, 128], bf16, tag="tp6")
                    nc.tensor.transpose(
                        ptp, nm_sb[:, t, fc * 128:(fc + 1) * 128], id_sb)
                    nc.scalar.copy(out=nmf[:, fc, :], in_=ptp)
                gate = s6_p.tile([128, 1024], bf16, tag="gate")
                for ng in range(2):
                    pg = pg_p.tile([128, 512], f32, tag="pg")
                    for fc in range(NFC):
                        nc.tensor.matmul(
                            pg, lhsT=naf[:, fc, :],
                            rhs=gw1_sb[:, fc, ng * 512:(ng + 1) * 512],
                            start=(fc == 0), stop=False)
                    for fc in range(NFC):
                        nc.tensor.matmul(
                            pg, lhsT=nmf[:, fc, :],
                            rhs=gw2_sb[:, fc, ng * 512:(ng + 1) * 512],
                            start=False, stop=False)
                    nc.tensor.matmul(
                        pg, lhsT=ones_sb[0:1, 0:128],
                        rhs=brow_sb[0:1, 7 * D + ng * 512:
                                    7 * D + (ng + 1) * 512],
                        start=False, stop=True)
                    nc.scalar.activation(
                        out=gate[:, ng * 512:(ng + 1) * 512], in_=pg,
                        func=AF.Sigmoid)
                diff = s6_p.tile([128, 1024], bf16, tag="diff")
                nc.vector.tensor_tensor(out=diff, in0=na_sb[:, t, :],
                                        in1=nm_sb[:, t, :], op=AL.subtract)
                nc.vector.tensor_mul(diff, diff, gate)
                of = s6_p.tile([128, 1024], f32, tag="of")
                nc.vector.tensor_tensor(out=of, in0=nm_sb[:, t, :],
                                        in1=diff, op=AL.add)
                nc.sync.dma_start(out=out[t * 128:(t + 1) * 128, :], in_=of)
                dna = s6_p.tile([128, 1024], f32, tag="dna")
                nc.scalar.copy(out=dna, in_=na_sb[:, t, :])
                nc.sync.dma_start(out=out_na[t * 128:(t + 1) * 128, :], in_=dna)
                dnm = s6_p.tile([128, 1024], f32, tag="dnm")
                nc.scalar.copy(out=dnm, in_=nm_sb[:, t, :])
                nc.sync.dma_start(out=out_nm[t * 128:(t + 1) * 128, :], in_=dnm)

    nc.compile()
    return nc


def _host_inputs(x, persistent_memory, Wq, bq, Wk, bk, Wv, bv, Wo, bo,
                 mWq, mbq, mWk, mbk, mWv, mbv,
                 g1_w, g1_b, g2_w, g2_b, gate_W, gate_b, window_size):
    win = int(window_size)
    f32 = np.float32
    combined = np.concatenate(
        [np.broadcast_to(np.asarray(persistent_memory, f32)[None], (B, PM, D)),
         np.asarray(x, f32)], axis=1)
    sHD = f32(1.0 / np.sqrt(HD))
    sD = f32(1.0 / np.sqrt(D))

    def b16(a):
        return np.ascontiguousarray(np.asarray(a, f32)).astype(BF16)

    shared = {
        "wq": b16(np.asarray(Wq, f32) * sHD), "wk": b16(Wk), "wv": b16(Wv),
        "wo": b16(Wo),
        "mwq": b16(np.asarray(mWq, f32) * sD),
        "mwk": b16(np.asarray(mWk, f32) * 0.1),
        "mwv": b16(np.asarray(mWv, f32) * 0.1),
        "gw1": b16(np.asarray(gate_W, f32)[:D]),
        "gw2": b16(np.asarray(gate_W, f32)[D:]),
        "brows": np.stack(
            [np.asarray(bq, f32) * sHD, np.asarray(bk, f32),
             np.asarray(bv, f32), np.asarray(bo, f32),
             np.asarray(mbq, f32) * sD, np.asarray(mbk, f32),
             np.asarray(mbv, f32), np.asarray(gate_b, f32)]
        ).astype(BF16).reshape(1, 8 * D),
        "gvecs": np.stack(
            [np.asarray(g1_w, f32), np.asarray(g1_b, f32),
             np.asarray(g2_w, f32), np.asarray(g2_b, f32)]).astype(BF16),
        "ident": np.eye(128, dtype=BF16),
        "onesr": np.ones((1, 512), dtype=BF16),
    }

    in_maps = []
    ki = np.arange(128)[:, None]
    qi = np.arange(128)[None, :]
    for c in range(NC_):
        b, h = c // 2, c % 2
        qs0 = h * QH
        lo = qs0 - WIN
        ck = np.zeros((KV, D), f32)
        src_lo = max(lo, 0)
        ck[src_lo - lo: KV] = combined[b][src_lo: qs0 + QH]
        m = np.zeros((128, NQT, 3, 128), f32)
        for t in range(NQT):
            for cc in range(3):
                kg = lo + (t + cc) * 128 + ki
                qg = qs0 + t * 128 + qi
                dd = qg - kg
                m[:, t, cc, :] = ((dd >= 0) & (dd <= win) & (kg >= 0))
        im = dict(shared)
        im["cfT"] = b16(combined[b].T)
        im["ckT"] = b16(ck.T)
        im["masks"] = m.reshape(128, NQT * 384).astype(BF16)
        in_maps.append(im)
    return in_maps


def kernel(**inputs):
    from concourse import bass_utils
    if "nc" not in _CACHE:
        _CACHE["nc"] = _build_program()
    nc = _CACHE["nc"]
    in_maps = _host_inputs(**inputs)
    res = bass_utils.run_bass_kernel_spmd(nc, in_maps, core_ids=list(range(NC_)))
    outp = np.zeros((B, S, D), np.float32)
    for c in range(NC_):
        b, h = c // 2, c % 2
        outp[b, h * QH:(h + 1) * QH] = res.results[c]["out"]
    return outp

